# revision 15
# baseline (speedup 1.0000x reference)
"""MoE expert-collection grouped GEMM for Trainium2, expert-parallel over 8
NeuronCores.

Problem (hardcoded shapes):
  sorted_features  [65536, 1024] f32   tokens sorted by expert, 4096/expert
  expert_ids_sorted[65536] i32         unused: split is static equal-count
  routing_matrix   [1024, 2048, 16] f32
  routing_bias     [2048, 16] f32
  out = silu(x_e @ W_e + b_e) per expert  -> [65536, 2048] f32

Sharding: expert-parallel, 2 experts (= 8192 contiguous sorted tokens) per
core. Host-side dispatch hands each core its token block transposed
(feature-major, fp8 e4m3) plus its 2 experts' weights (fp8 e4m3, pre-scaled
x128 so w_std 0.0054 lands in e4m3's normal range) and bias pre-broadcast to
128 partitions (fp32, pre-scaled x128 to match).

Device pipeline per core: 1024 fp8 DoubleRow matmuls (K=256 per instruction,
2x PE throughput vs fp16) accumulating in fp32 PSUM (t-on-partitions x
o-free tiles, contraction over 4 k-pair blocks), DVE bias add (in fp32 x128
domain, fp16 out), ACT Silu with scale=1/128 folding the weight scale back
out (fp16 out), fp16 store. x loads ride the SP HWDGE ring; weight loads and
output stores ride the ACT HWDGE ring.
"""

import ml_dtypes
import numpy as np

import concourse.bass as bass
import concourse.mybir as mybir
import concourse.tile as tile
from concourse.bass_utils import run_bass_kernel_spmd

N_CORES = 8
N_TOKENS = 65536
D_IN = 1024
D_OUT = 2048
N_EXPERTS = 16
E_PER_CORE = N_EXPERTS // N_CORES        # 2
TOK_PER_CORE = N_TOKENS // N_CORES       # 8192
TOK_PER_EXPERT = N_TOKENS // N_EXPERTS   # 4096

P = 128
KB = D_IN // P            # 8 contraction blocks
TS = 512                  # token stripe
OB = 512                  # out-feature block (one PSUM bank)
N_OB = D_OUT // OB        # 4
N_TSUB = TS // P          # 4
STRIPES_PER_EXPERT = TOK_PER_EXPERT // TS  # 8

F32 = mybir.dt.float32
F16 = mybir.dt.float16
F8 = mybir.dt.float8e4
NP_F8 = ml_dtypes.float8_e4m3
W_SCALE = 128.0  # lifts w_std ~0.0054 out of e4m3 subnormal territory


def _split_multi_waits(nc):
    """This container's walrus encodes at most ONE sync-wait per instruction;
    hoist extras onto single-wait NoOps inserted just before, same engine."""
    for fn in nc.m.functions:
        for bb in fn.blocks:
            insts = list(bb.instructions)
            out = []
            dirty = False
            for inst in insts:
                si = inst.sync_info
                waits = list(si.on_wait) if si and si.on_wait else []
                if len(waits) > 1:
                    dirty = True
                    for j, w in enumerate(waits[:-1]):
                        nop = mybir.InstNoOp(
                            name=f"{inst.name}-prewait{j}", ins=[], outs=[]
                        )
                        nop.engine = inst.engine
                        nop.sync_info = mybir.SyncInfo(on_wait=[w], on_update=[])
                        out.append(nop)
                    inst.sync_info = mybir.SyncInfo(
                        on_wait=[waits[-1]],
                        on_update=list(si.on_update) if si.on_update else [],
                    )
                out.append(inst)
            if dirty:
                bb.instructions = out


N_STRIPES = E_PER_CORE * STRIPES_PER_EXPERT  # 16


def build_kernel():
    nc = bass.Bass()
    # xt pre-striped on host: [stripe, partition, kb, t] so each stripe loads
    # with 8KB-contiguous per-partition lines
    xt = nc.dram_tensor("xt", [N_STRIPES, P, KB, TS], F8, kind="ExternalInput")
    w = nc.dram_tensor("w", [E_PER_CORE, D_IN, D_OUT], F8, kind="ExternalInput")
    bb = nc.dram_tensor("bb", [E_PER_CORE, P, D_OUT], F32, kind="ExternalInput")
    y = nc.dram_tensor("y", [TOK_PER_CORE, D_OUT], F16, kind="ExternalOutput")

    with tile.TileContext(nc) as tc:
        with (
            tc.tile_pool(name="persist", bufs=1) as persist,
            tc.tile_pool(name="xp", bufs=3) as xp,
            tc.tile_pool(name="outs", bufs=3) as outs,
            tc.tile_pool(name="psum", bufs=8, space="PSUM") as psump,
        ):
            # stripe-0 x rides the SP ring in kb-pair slices so the first
            # LDWEIGHTS only waits on 128KB, not the full 512KB stripe
            x16_tiles = {}
            x16_tiles[0] = xp.tile([P, KB, TS], F8, tag="x16", name="x16_s0")
            for h in range(KB // 2):
                nc.sync.dma_start(
                    x16_tiles[0][:, 2 * h:2 * h + 2, :],
                    xt[0][:, 2 * h:2 * h + 2, :],
                )

            # two half-kb W tiles per expert, one big DMA each on its own
            # HWDGE ring: full 4KB contiguous lines, no shared-tile writes,
            # no DMA-sem recycle chains. Expert 1's weights + bias are
            # DEFERRED past stripe 0 so the critical preload (x0 + W e0) gets
            # the full pair-shared HBM bandwidth.
            KH = 2  # kb per W tile -> 4 tiles/expert, ~1MB DMAs
            NWT = KB // KH
            b_sb = [
                persist.tile([P, D_OUT], F32, name=f"bias_{e}")
                for e in range(E_PER_CORE)
            ]
            w16 = [
                [
                    persist.tile([P, KH, D_OUT], F8, name=f"w16_{e}_{h}")
                    for h in range(NWT)
                ]
                for e in range(E_PER_CORE)
            ]

            def load_expert(e):
                w_src = w[e].rearrange("(kb p) o -> p kb o", p=P)
                for h in range(NWT):
                    eng = nc.scalar if h % 2 == 0 else nc.sync
                    eng.dma_start(w16[e][h][:], w_src[:, h * KH:(h + 1) * KH, :])
                nc.gpsimd.dma_start(b_sb[e][:], bb[e])

            def load_expert0():
                # expert 0 gates the ramp: all 2MB of it is needed before the
                # first PSUM group completes. Slice it into o-halves issued in
                # need-order (ob 0/1 pieces of every kb-pair first) across the
                # scalar and gpsimd rings so matmuls start after ~256KB has
                # landed instead of 2MB; all queues share one HBM pipe, so
                # granularity (not queue count) is what shortens the stall.
                w_src = w[0].rearrange("(kb p) o -> p kb o", p=P)
                nc.gpsimd.dma_start(b_sb[0][:], bb[0])
                half = D_OUT // 2
                for os_ in range(2):
                    sl = slice(os_ * half, (os_ + 1) * half)
                    for h in range(NWT):
                        eng = nc.scalar if h % 2 == 0 else nc.gpsimd
                        eng.dma_start(
                            w16[0][h][:, :, sl],
                            w_src[:, h * KH:(h + 1) * KH, sl],
                        )

            load_expert0()

            for e in range(E_PER_CORE):
                for s in range(STRIPES_PER_EXPERT):
                    g = e * STRIPES_PER_EXPERT + s
                    t0 = g * TS
                    if g in x16_tiles:
                        x16 = x16_tiles[g]
                    else:
                        x16 = xp.tile([P, KB, TS], F8, tag="x16", name="x16")
                        nc.sync.dma_start(x16[:], xt[g])

                    for tsub in range(N_TSUB):
                        last = g == N_STRIPES - 1 and tsub == N_TSUB - 1
                        y_pre = outs.tile([P, D_OUT], F16, tag="ypre")
                        y_act = outs.tile([P, D_OUT], F16, tag="yact")
                        for ob in range(N_OB):
                            ps = psump.tile([P, OB], F32, tag="ps")
                            for h in range(NWT):
                                # DoubleRow: K=256 (one kb pair) per matmul
                                nc.tensor.matmul(
                                    ps[:],
                                    lhsT=x16[
                                        :, 2 * h:2 * h + 2,
                                        tsub * P:(tsub + 1) * P,
                                    ],
                                    rhs=w16[e][h][:, :, ob * OB:(ob + 1) * OB],
                                    start=(h == 0),
                                    stop=(h == NWT - 1),
                                    perf_mode=mybir.MatmulPerfMode.DoubleRow,
                                )
                            # bias add in the x128 domain (bias pre-scaled on
                            # host); fp16 out is exact enough at |v|<~700
                            nc.vector.tensor_tensor(
                                y_pre[:, ob * OB:(ob + 1) * OB], ps[:],
                                b_sb[e][:, ob * OB:(ob + 1) * OB],
                                mybir.AluOpType.add,
                            )
                            if last:
                                # final tile: silu+store per-ob so the tail
                                # chain after the last matmul stays short
                                nc.scalar.activation(
                                    y_act[:, ob * OB:(ob + 1) * OB],
                                    y_pre[:, ob * OB:(ob + 1) * OB],
                                    mybir.ActivationFunctionType.Silu,
                                    scale=1.0 / W_SCALE,
                                )
                                nc.gpsimd.dma_start(
                                    y[t0 + tsub * P:t0 + (tsub + 1) * P,
                                      ob * OB:(ob + 1) * OB],
                                    y_act[:, ob * OB:(ob + 1) * OB],
                                )
                        if not last:
                            # one fused silu per 2048-wide tile amortizes the
                            # ~300ns fixed ACT cost; scale folds the x128
                            # weight scale back out before the nonlinearity
                            nc.scalar.activation(
                                y_act[:], y_pre[:],
                                mybir.ActivationFunctionType.Silu,
                                scale=1.0 / W_SCALE,
                            )
                            nc.gpsimd.dma_start(
                                y[t0 + tsub * P:t0 + (tsub + 1) * P, :], y_act[:]
                            )
                    if g == 2:
                        # deferred past the ramp window so expert 1's 2MB
                        # doesn't steal HBM bandwidth from the critical
                        # expert-0 + stripe-0 loads (needed from stripe 8,
                        # ~100us later)
                        load_expert(1)

    _split_multi_waits(nc)
    return nc


_NC_CACHE = None


def _get_nc():
    global _NC_CACHE
    if _NC_CACHE is None:
        _NC_CACHE = build_kernel()
    return _NC_CACHE


def _in_maps(sorted_features, routing_matrix, routing_bias):
    maps = []
    for c in range(N_CORES):
        rows = slice(c * TOK_PER_CORE, (c + 1) * TOK_PER_CORE)
        es = slice(c * E_PER_CORE, (c + 1) * E_PER_CORE)
        # [stripe, partition, kb, t]: element (s,p,kb,t) = X_c[s*TS+t, kb*P+p]
        xt_c = np.ascontiguousarray(
            sorted_features[rows]
            .reshape(N_STRIPES, TS, KB, P)
            .transpose(0, 3, 2, 1)
            .astype(NP_F8)
        )
        w_c = np.ascontiguousarray(
            (routing_matrix[:, :, es].transpose(2, 0, 1) * W_SCALE).astype(NP_F8)
        )
        # bias enters the DVE add in the x128 domain: silu((ps + S*b)/S)
        b_c = np.ascontiguousarray(
            np.broadcast_to(
                (routing_bias[:, es].T * W_SCALE)[:, None, :],
                (E_PER_CORE, P, D_OUT),
            ).astype(np.float32)
        )
        maps.append({"xt": xt_c, "w": w_c, "bb": b_c})
    return maps


def run(sorted_features, routing_matrix, routing_bias, **run_kwargs):
    nc = _get_nc()
    maps = _in_maps(sorted_features, routing_matrix, routing_bias)
    res = run_bass_kernel_spmd(nc, maps, core_ids=list(range(N_CORES)), **run_kwargs)
    out = np.concatenate(
        [res.results[c]["y"].astype(np.float32) for c in range(N_CORES)], axis=0
    )
    return out, res


def kernel(sorted_features, expert_ids_sorted, routing_matrix, routing_bias):
    assert sorted_features.shape == (N_TOKENS, D_IN)
    assert routing_matrix.shape == (D_IN, D_OUT, N_EXPERTS)
    assert routing_bias.shape == (D_OUT, N_EXPERTS)
    out, _ = run(
        np.asarray(sorted_features, dtype=np.float32),
        np.asarray(routing_matrix, dtype=np.float32),
        np.asarray(routing_bias, dtype=np.float32),
    )
    return out



# revision 21
# speedup vs baseline: 1.0188x; 1.0188x over previous
"""MoE expert-collection grouped GEMM for Trainium2, expert-parallel over 8
NeuronCores.

Problem (hardcoded shapes):
  sorted_features  [65536, 1024] f32   tokens sorted by expert, 4096/expert
  expert_ids_sorted[65536] i32         unused: split is static equal-count
  routing_matrix   [1024, 2048, 16] f32
  routing_bias     [2048, 16] f32
  out = silu(x_e @ W_e + b_e) per expert  -> [65536, 2048] f32

Sharding: expert-parallel, 2 experts (= 8192 contiguous sorted tokens) per
core. Host-side dispatch hands each core its token block transposed
(feature-major, fp8 e4m3) plus its 2 experts' weights (fp8 e4m3, pre-scaled
x128 so w_std 0.0054 lands in e4m3's normal range) and bias pre-broadcast to
128 partitions (fp32, pre-scaled x128 to match).

Device pipeline per core: 1024 fp8 DoubleRow matmuls (K=256 per instruction,
2x PE throughput vs fp16) accumulating in fp32 PSUM (t-on-partitions x
o-free tiles, contraction over 4 k-pair blocks), DVE bias add (in fp32 x128
domain, fp16 out), ACT Silu with scale=1/128 folding the weight scale back
out (fp16 out), fp16 store. x loads ride the SP HWDGE ring; weight loads and
output stores ride the ACT HWDGE ring.
"""

import ml_dtypes
import numpy as np

import concourse.bass as bass
import concourse.mybir as mybir
import concourse.tile as tile
from concourse.bass_utils import run_bass_kernel_spmd

N_CORES = 8
N_TOKENS = 65536
D_IN = 1024
D_OUT = 2048
N_EXPERTS = 16
E_PER_CORE = N_EXPERTS // N_CORES        # 2
TOK_PER_CORE = N_TOKENS // N_CORES       # 8192
TOK_PER_EXPERT = N_TOKENS // N_EXPERTS   # 4096

P = 128
KB = D_IN // P            # 8 contraction blocks
TS = 512                  # token stripe
OB = 512                  # out-feature block (one PSUM bank)
N_OB = D_OUT // OB        # 4
N_TSUB = TS // P          # 4
STRIPES_PER_EXPERT = TOK_PER_EXPERT // TS  # 8

F32 = mybir.dt.float32
F16 = mybir.dt.float16
F8 = mybir.dt.float8e4
NP_F8 = ml_dtypes.float8_e4m3
W_SCALE = 128.0  # lifts w_std ~0.0054 out of e4m3 subnormal territory
KH_G = 2          # kb per W tile = one DoubleRow k-pair
NWT_G = KB // KH_G  # 4 W tiles per expert


def _split_multi_waits(nc):
    """This container's walrus encodes at most ONE sync-wait per instruction;
    hoist extras onto single-wait NoOps inserted just before, same engine."""
    for fn in nc.m.functions:
        for bb in fn.blocks:
            insts = list(bb.instructions)
            out = []
            dirty = False
            for inst in insts:
                si = inst.sync_info
                waits = list(si.on_wait) if si and si.on_wait else []
                if len(waits) > 1:
                    dirty = True
                    for j, w in enumerate(waits[:-1]):
                        nop = mybir.InstNoOp(
                            name=f"{inst.name}-prewait{j}", ins=[], outs=[]
                        )
                        nop.engine = inst.engine
                        nop.sync_info = mybir.SyncInfo(on_wait=[w], on_update=[])
                        out.append(nop)
                    inst.sync_info = mybir.SyncInfo(
                        on_wait=[waits[-1]],
                        on_update=list(si.on_update) if si.on_update else [],
                    )
                out.append(inst)
            if dirty:
                bb.instructions = out


N_STRIPES = E_PER_CORE * STRIPES_PER_EXPERT  # 16


def build_kernel():
    nc = bass.Bass()
    # xt pre-striped on host: [stripe, partition, kb, t] so each stripe loads
    # with 8KB-contiguous per-partition lines
    xt = nc.dram_tensor("xt", [N_STRIPES, P, KB, TS], F8, kind="ExternalInput")
    # w pre-packed on host into tile layout, o-halved: [e, h, os, p, kh, o']
    # so every W DMA is a fully contiguous per-partition read (the naive
    # "(kb p) o" rearrange reads scattered 2KB chunks at ~1/4 bandwidth)
    w = nc.dram_tensor(
        "w", [E_PER_CORE, NWT_G, 2, P, KH_G, D_OUT // 2], F8, kind="ExternalInput"
    )
    bb = nc.dram_tensor("bb", [E_PER_CORE, P, D_OUT], F32, kind="ExternalInput")
    y = nc.dram_tensor("y", [TOK_PER_CORE, D_OUT], F16, kind="ExternalOutput")

    with tile.TileContext(nc) as tc:
        with (
            tc.tile_pool(name="persist", bufs=1) as persist,
            tc.tile_pool(name="xp", bufs=3) as xp,
            tc.tile_pool(name="outs", bufs=3) as outs,
            tc.tile_pool(name="psum", bufs=8, space="PSUM") as psump,
        ):
            # stripe-0 x rides the SP ring in kb-pair slices so the first
            # LDWEIGHTS only waits on 128KB, not the full 512KB stripe
            x16_tiles = {}
            x16_tiles[0] = xp.tile([P, KB, TS], F8, tag="x16", name="x16_s0")
            for h in range(KB // 2):
                nc.sync.dma_start(
                    x16_tiles[0][:, 2 * h:2 * h + 2, :],
                    xt[0][:, 2 * h:2 * h + 2, :],
                )

            KH = KH_G
            NWT = NWT_G
            b_sb = [
                persist.tile([P, D_OUT], F32, name=f"bias_{e}")
                for e in range(E_PER_CORE)
            ]
            w16 = [
                [
                    persist.tile([P, KH, D_OUT], F8, name=f"w16_{e}_{h}")
                    for h in range(NWT)
                ]
                for e in range(E_PER_CORE)
            ]

            half = D_OUT // 2

            def load_expert(e, engs):
                # 8 contiguous 256KB DMAs per expert, need-ordered (os outer,
                # h inner) so the first accumulation groups unblock after
                # ~256KB instead of the full 2MB; all rings share one HBM
                # pipe, so granularity (not queue count) is what matters.
                nc.gpsimd.dma_start(b_sb[e][:], bb[e])
                for os_ in range(2):
                    sl = slice(os_ * half, (os_ + 1) * half)
                    for h in range(NWT):
                        engs[h % len(engs)].dma_start(
                            w16[e][h][:, :, sl], w[e, h, os_]
                        )

            load_expert(0, [nc.scalar, nc.gpsimd])

            for e in range(E_PER_CORE):
                for s in range(STRIPES_PER_EXPERT):
                    g = e * STRIPES_PER_EXPERT + s
                    t0 = g * TS
                    if g in x16_tiles:
                        x16 = x16_tiles[g]
                    else:
                        x16 = xp.tile([P, KB, TS], F8, tag="x16", name="x16")
                        nc.sync.dma_start(x16[:], xt[g])

                    for tsub in range(N_TSUB):
                        last = g == N_STRIPES - 1 and tsub == N_TSUB - 1
                        y_pre = outs.tile([P, D_OUT], F16, tag="ypre")
                        y_act = outs.tile([P, D_OUT], F16, tag="yact")
                        for ob in range(N_OB):
                            ps = psump.tile([P, OB], F32, tag="ps")
                            for h in range(NWT):
                                # DoubleRow: K=256 (one kb pair) per matmul
                                nc.tensor.matmul(
                                    ps[:],
                                    lhsT=x16[
                                        :, 2 * h:2 * h + 2,
                                        tsub * P:(tsub + 1) * P,
                                    ],
                                    rhs=w16[e][h][:, :, ob * OB:(ob + 1) * OB],
                                    start=(h == 0),
                                    stop=(h == NWT - 1),
                                    perf_mode=mybir.MatmulPerfMode.DoubleRow,
                                )
                            # bias add in the x128 domain (bias pre-scaled on
                            # host); fp16 out is exact enough at |v|<~700
                            nc.vector.tensor_tensor(
                                y_pre[:, ob * OB:(ob + 1) * OB], ps[:],
                                b_sb[e][:, ob * OB:(ob + 1) * OB],
                                mybir.AluOpType.add,
                            )
                            if last:
                                # final tile: silu+store per-ob so the tail
                                # chain after the last matmul stays short
                                nc.scalar.activation(
                                    y_act[:, ob * OB:(ob + 1) * OB],
                                    y_pre[:, ob * OB:(ob + 1) * OB],
                                    mybir.ActivationFunctionType.Silu,
                                    scale=1.0 / W_SCALE,
                                )
                                nc.gpsimd.dma_start(
                                    y[t0 + tsub * P:t0 + (tsub + 1) * P,
                                      ob * OB:(ob + 1) * OB],
                                    y_act[:, ob * OB:(ob + 1) * OB],
                                )
                        if not last:
                            # one fused silu per 2048-wide tile amortizes the
                            # ~300ns fixed ACT cost; scale folds the x128
                            # weight scale back out before the nonlinearity
                            nc.scalar.activation(
                                y_act[:], y_pre[:],
                                mybir.ActivationFunctionType.Silu,
                                scale=1.0 / W_SCALE,
                            )
                            nc.gpsimd.dma_start(
                                y[t0 + tsub * P:t0 + (tsub + 1) * P, :], y_act[:]
                            )
                    if g == 0:
                        load_expert(1, [nc.scalar, nc.sync])

    _split_multi_waits(nc)
    return nc


_NC_CACHE = None


def _get_nc():
    global _NC_CACHE
    if _NC_CACHE is None:
        _NC_CACHE = build_kernel()
    return _NC_CACHE


def _in_maps(sorted_features, routing_matrix, routing_bias):
    maps = []
    for c in range(N_CORES):
        rows = slice(c * TOK_PER_CORE, (c + 1) * TOK_PER_CORE)
        es = slice(c * E_PER_CORE, (c + 1) * E_PER_CORE)
        # [stripe, partition, kb, t]: element (s,p,kb,t) = X_c[s*TS+t, kb*P+p]
        xt_c = np.ascontiguousarray(
            sorted_features[rows]
            .reshape(N_STRIPES, TS, KB, P)
            .transpose(0, 3, 2, 1)
            .astype(NP_F8)
        )
        # pack into the device tile layout [e, h, os, p, kh, o'] so each W
        # DMA reads a fully contiguous 256KB block: kin = (h*KH+kh)*128 + p
        w_c = np.ascontiguousarray(
            (routing_matrix[:, :, es].transpose(2, 0, 1) * W_SCALE)
            .astype(NP_F8)
            .reshape(E_PER_CORE, NWT_G, KH_G, P, 2, D_OUT // 2)
            .transpose(0, 1, 4, 3, 2, 5)
        )
        # bias enters the DVE add in the x128 domain: silu((ps + S*b)/S)
        b_c = np.ascontiguousarray(
            np.broadcast_to(
                (routing_bias[:, es].T * W_SCALE)[:, None, :],
                (E_PER_CORE, P, D_OUT),
            ).astype(np.float32)
        )
        maps.append({"xt": xt_c, "w": w_c, "bb": b_c})
    return maps


def run(sorted_features, routing_matrix, routing_bias, **run_kwargs):
    nc = _get_nc()
    maps = _in_maps(sorted_features, routing_matrix, routing_bias)
    res = run_bass_kernel_spmd(nc, maps, core_ids=list(range(N_CORES)), **run_kwargs)
    out = np.concatenate(
        [res.results[c]["y"].astype(np.float32) for c in range(N_CORES)], axis=0
    )
    return out, res


def kernel(sorted_features, expert_ids_sorted, routing_matrix, routing_bias):
    assert sorted_features.shape == (N_TOKENS, D_IN)
    assert routing_matrix.shape == (D_IN, D_OUT, N_EXPERTS)
    assert routing_bias.shape == (D_OUT, N_EXPERTS)
    out, _ = run(
        np.asarray(sorted_features, dtype=np.float32),
        np.asarray(routing_matrix, dtype=np.float32),
        np.asarray(routing_bias, dtype=np.float32),
    )
    return out



# revision 35
# speedup vs baseline: 1.0408x; 1.0217x over previous
"""MoE expert-collection grouped GEMM for Trainium2, expert-parallel over 8
NeuronCores.

Problem (hardcoded shapes):
  sorted_features  [65536, 1024] f32   tokens sorted by expert, 4096/expert
  expert_ids_sorted[65536] i32         unused: split is static equal-count
  routing_matrix   [1024, 2048, 16] f32
  routing_bias     [2048, 16] f32
  out = silu(x_e @ W_e + b_e) per expert  -> [65536, 2048] f32

Sharding: expert-parallel, 2 experts (= 8192 contiguous sorted tokens) per
core. Host-side dispatch hands each core its token block transposed
(feature-major, fp8 e4m3) plus its 2 experts' weights (fp8 e4m3, pre-scaled
x128 so w_std 0.0054 lands in e4m3's normal range) and bias pre-broadcast to
128 partitions (fp32, pre-scaled x128 to match).

Device pipeline per core: 1024 fp8 DoubleRow matmuls (K=256 per instruction,
2x PE throughput vs fp16) accumulating in fp32 PSUM (t-on-partitions x
o-free tiles, contraction over 4 k-pair blocks), DVE bias add (in fp32 x128
domain, fp16 out), ACT Silu with scale=1/128 folding the weight scale back
out (fp16 out), fp16 store. x loads ride the SP HWDGE ring; weight loads and
output stores ride the ACT HWDGE ring.
"""

import ml_dtypes
import numpy as np

import concourse.bass as bass
import concourse.mybir as mybir
import concourse.tile as tile
from concourse.bass_utils import run_bass_kernel_spmd

N_CORES = 8
N_TOKENS = 65536
D_IN = 1024
D_OUT = 2048
N_EXPERTS = 16
E_PER_CORE = N_EXPERTS // N_CORES        # 2
TOK_PER_CORE = N_TOKENS // N_CORES       # 8192
TOK_PER_EXPERT = N_TOKENS // N_EXPERTS   # 4096

P = 128
KB = D_IN // P            # 8 contraction blocks
TS = 512                  # token stripe
OB = 512                  # out-feature block (one PSUM bank)
N_OB = D_OUT // OB        # 4
N_TSUB = TS // P          # 4
STRIPES_PER_EXPERT = TOK_PER_EXPERT // TS  # 8

F32 = mybir.dt.float32
F16 = mybir.dt.float16
F8 = mybir.dt.float8e4
NP_F8 = ml_dtypes.float8_e4m3
W_SCALE = 128.0  # lifts w_std ~0.0054 out of e4m3 subnormal territory
KH_G = 2          # kb per W tile = one DoubleRow k-pair
NWT_G = KB // KH_G  # 4 W tiles per expert


def _split_multi_waits(nc):
    """This container's walrus encodes at most ONE sync-wait per instruction;
    hoist extras onto single-wait NoOps inserted just before, same engine."""
    for fn in nc.m.functions:
        for bb in fn.blocks:
            insts = list(bb.instructions)
            out = []
            dirty = False
            for inst in insts:
                si = inst.sync_info
                waits = list(si.on_wait) if si and si.on_wait else []
                if len(waits) > 1:
                    dirty = True
                    for j, w in enumerate(waits[:-1]):
                        nop = mybir.InstNoOp(
                            name=f"{inst.name}-prewait{j}", ins=[], outs=[]
                        )
                        nop.engine = inst.engine
                        nop.sync_info = mybir.SyncInfo(on_wait=[w], on_update=[])
                        out.append(nop)
                    inst.sync_info = mybir.SyncInfo(
                        on_wait=[waits[-1]],
                        on_update=list(si.on_update) if si.on_update else [],
                    )
                out.append(inst)
            if dirty:
                bb.instructions = out


N_STRIPES = E_PER_CORE * STRIPES_PER_EXPERT  # 16


def build_kernel():
    nc = bass.Bass()
    # xt pre-striped on host: [stripe, partition, kb, t] so each stripe loads
    # with 8KB-contiguous per-partition lines
    xt = nc.dram_tensor("xt", [N_STRIPES, P, KB, TS], F8, kind="ExternalInput")
    # w pre-packed on host into the exact sbuf tile layout [e, h, p, os, kh, o']
    # so W DMAs are fully contiguous per-partition reads with 4KB (full-tile)
    # or 2KB (os-half) elements — the naive "(kb p) o" rearrange reads
    # scattered 1-2KB chunks at a fraction of the per-queue bandwidth
    w = nc.dram_tensor(
        "w", [E_PER_CORE, NWT_G, P, 2, KH_G, D_OUT // 2], F8, kind="ExternalInput"
    )
    # bias pre-broadcast on host in fp16 (512KB/expert; fp32 was 1MB of
    # redundant DMA sitting in front of ramp-critical W slices, and the
    # on-device partition_broadcast op doesn't encode in this toolchain)
    bb = nc.dram_tensor("bb", [E_PER_CORE, P, D_OUT], F16, kind="ExternalInput")
    y = nc.dram_tensor("y", [TOK_PER_CORE, D_OUT], F16, kind="ExternalOutput")

    with tile.TileContext(nc) as tc:
        with (
            tc.tile_pool(name="persist", bufs=1) as persist,
            tc.tile_pool(name="xp", bufs=3) as xp,
            tc.tile_pool(name="outs", bufs=3) as outs,
            tc.tile_pool(name="psum", bufs=8, space="PSUM") as psump,
        ):
            x16_tiles = {}
            x16_tiles[0] = xp.tile([P, KB, TS], F8, tag="x16", name="x16_s0")
            nc.sync.dma_start(x16_tiles[0][:], xt[0])

            KH = KH_G
            NWT = NWT_G
            b_sb = [
                persist.tile([P, D_OUT], F16, name=f"bias_{e}")
                for e in range(E_PER_CORE)
            ]
            # os-major W tiles: [p, os, kh, o'] so an os-half is a contiguous
            # 2KB run per partition (DMA elem size drives queue bandwidth)
            w16 = [
                [
                    persist.tile([P, 2, KH, D_OUT // 2], F8, name=f"w16_{e}_{h}")
                    for h in range(NWT)
                ]
                for e in range(E_PER_CORE)
            ]

            half = D_OUT // 2

            def load_expert0():
                # expert 0 gates the ramp: slice into need-ordered 256KB
                # os-halves across two rings so the first accumulation groups
                # unblock early; each queue tops out well under the core's
                # aggregate HBM rate, so splitting rings + large elements is
                # what shortens the stall. Bias halves slot in right after
                # the os0 W pieces (first DVE read comes ~2us after the
                # first matmul, with 8 PSUM banks of runway).
                for h in range(NWT):
                    eng = nc.scalar if h % 2 == 0 else nc.gpsimd
                    eng.dma_start(w16[0][h][:, 0], w[0, h, :, 0])
                nc.gpsimd.dma_start(b_sb[0][:, :half], bb[0][:, :half])
                nc.sync.dma_start(b_sb[0][:, half:], bb[0][:, half:])
                for h in range(NWT):
                    eng = nc.scalar if h % 2 == 0 else nc.gpsimd
                    eng.dma_start(w16[0][h][:, 1], w[0, h, :, 1])

            def load_expert1():
                # mid-flight, off the critical path: full-tile 512KB DMAs
                # with 4KB elements
                nc.gpsimd.dma_start(b_sb[1][:], bb[1])
                for h in range(NWT):
                    eng = nc.scalar if h % 2 == 0 else nc.sync
                    eng.dma_start(w16[1][h][:], w[1, h])

            load_expert0()

            for e in range(E_PER_CORE):
                for s in range(STRIPES_PER_EXPERT):
                    g = e * STRIPES_PER_EXPERT + s
                    t0 = g * TS
                    if g in x16_tiles:
                        x16 = x16_tiles[g]
                    else:
                        x16 = xp.tile([P, KB, TS], F8, tag="x16", name="x16")
                        nc.sync.dma_start(x16[:], xt[g])

                    for tsub in range(N_TSUB):
                        last = g == N_STRIPES - 1 and tsub == N_TSUB - 1
                        store_eng = [nc.gpsimd, nc.sync, nc.scalar][
                            (g * N_TSUB + tsub) % 3
                        ]
                        y_pre = outs.tile([P, D_OUT], F16, tag="ypre")
                        y_act = outs.tile([P, D_OUT], F16, tag="yact")
                        for ob in range(N_OB):
                            os_, oc = divmod(ob, 2)
                            ps = psump.tile([P, OB], F32, tag="ps")
                            for h in range(NWT):
                                # DoubleRow: K=256 (one kb pair) per matmul
                                nc.tensor.matmul(
                                    ps[:],
                                    lhsT=x16[
                                        :, 2 * h:2 * h + 2,
                                        tsub * P:(tsub + 1) * P,
                                    ],
                                    rhs=w16[e][h][
                                        :, os_, :, oc * OB:(oc + 1) * OB
                                    ],
                                    start=(h == 0),
                                    stop=(h == NWT - 1),
                                    perf_mode=mybir.MatmulPerfMode.DoubleRow,
                                )
                            # bias add in the x128 domain (bias pre-scaled on
                            # host); fp16 out is exact enough at |v|<~700
                            nc.vector.tensor_tensor(
                                y_pre[:, ob * OB:(ob + 1) * OB], ps[:],
                                b_sb[e][:, ob * OB:(ob + 1) * OB],
                                mybir.AluOpType.add,
                            )
                            if last:
                                # final tile: silu+store per-ob so the tail
                                # chain after the last matmul stays short
                                nc.scalar.activation(
                                    y_act[:, ob * OB:(ob + 1) * OB],
                                    y_pre[:, ob * OB:(ob + 1) * OB],
                                    mybir.ActivationFunctionType.Silu,
                                    scale=1.0 / W_SCALE,
                                )
                                [nc.gpsimd, nc.sync, nc.scalar][ob % 3].dma_start(
                                    y[t0 + tsub * P:t0 + (tsub + 1) * P,
                                      ob * OB:(ob + 1) * OB],
                                    y_act[:, ob * OB:(ob + 1) * OB],
                                )
                        if not last:
                            # one fused silu per 2048-wide tile amortizes the
                            # ~300ns fixed ACT cost; scale folds the x128
                            # weight scale back out before the nonlinearity
                            nc.scalar.activation(
                                y_act[:], y_pre[:],
                                mybir.ActivationFunctionType.Silu,
                                scale=1.0 / W_SCALE,
                            )
                            store_eng.dma_start(
                                y[t0 + tsub * P:t0 + (tsub + 1) * P, :], y_act[:]
                            )
                    if g == 1:
                        # after g==1 so expert 1's 1MB on the sync ring sits
                        # behind the already-enqueued x1/x2 prefetches
                        load_expert1()

    _split_multi_waits(nc)
    return nc


_NC_CACHE = None


def _get_nc():
    global _NC_CACHE
    if _NC_CACHE is None:
        _NC_CACHE = build_kernel()
    return _NC_CACHE


def _in_maps(sorted_features, routing_matrix, routing_bias):
    maps = []
    for c in range(N_CORES):
        rows = slice(c * TOK_PER_CORE, (c + 1) * TOK_PER_CORE)
        es = slice(c * E_PER_CORE, (c + 1) * E_PER_CORE)
        # [stripe, partition, kb, t]: element (s,p,kb,t) = X_c[s*TS+t, kb*P+p]
        xt_c = np.ascontiguousarray(
            sorted_features[rows]
            .reshape(N_STRIPES, TS, KB, P)
            .transpose(0, 3, 2, 1)
            .astype(NP_F8)
        )
        # pack into the device tile layout [e, h, p, os, kh, o'] so each W
        # DMA reads fully contiguous blocks: kin = (h*KH+kh)*128 + p
        w_c = np.ascontiguousarray(
            (routing_matrix[:, :, es].transpose(2, 0, 1) * W_SCALE)
            .astype(NP_F8)
            .reshape(E_PER_CORE, NWT_G, KH_G, P, 2, D_OUT // 2)
            .transpose(0, 1, 3, 4, 2, 5)
        )
        # bias enters the DVE add in the x128 domain: silu((ps + S*b)/S);
        # fp16 is exact to ~2^-11 relative, far under the fp8 matmul noise
        b_c = np.ascontiguousarray(
            np.broadcast_to(
                (routing_bias[:, es].T * W_SCALE)[:, None, :],
                (E_PER_CORE, P, D_OUT),
            ).astype(np.float16)
        )
        maps.append({"xt": xt_c, "w": w_c, "bb": b_c})
    return maps


def run(sorted_features, routing_matrix, routing_bias, **run_kwargs):
    nc = _get_nc()
    maps = _in_maps(sorted_features, routing_matrix, routing_bias)
    res = run_bass_kernel_spmd(nc, maps, core_ids=list(range(N_CORES)), **run_kwargs)
    out = np.concatenate(
        [res.results[c]["y"].astype(np.float32) for c in range(N_CORES)], axis=0
    )
    return out, res


def kernel(sorted_features, expert_ids_sorted, routing_matrix, routing_bias):
    assert sorted_features.shape == (N_TOKENS, D_IN)
    assert routing_matrix.shape == (D_IN, D_OUT, N_EXPERTS)
    assert routing_bias.shape == (D_OUT, N_EXPERTS)
    out, _ = run(
        np.asarray(sorted_features, dtype=np.float32),
        np.asarray(routing_matrix, dtype=np.float32),
        np.asarray(routing_bias, dtype=np.float32),
    )
    return out



# revision 36
# speedup vs baseline: 1.0581x; 1.0166x over previous
"""MoE expert-collection grouped GEMM for Trainium2, expert-parallel over 8
NeuronCores.

Problem (hardcoded shapes):
  sorted_features  [65536, 1024] f32   tokens sorted by expert, 4096/expert
  expert_ids_sorted[65536] i32         unused: split is static equal-count
  routing_matrix   [1024, 2048, 16] f32
  routing_bias     [2048, 16] f32
  out = silu(x_e @ W_e + b_e) per expert  -> [65536, 2048] f32

Sharding: expert-parallel, 2 experts (= 8192 contiguous sorted tokens) per
core. Host-side dispatch hands each core its token block transposed
(feature-major, fp8 e4m3) plus its 2 experts' weights (fp8 e4m3, pre-scaled
x128 so w_std 0.0054 lands in e4m3's normal range) and bias pre-broadcast to
128 partitions (fp32, pre-scaled x128 to match).

Device pipeline per core: 1024 fp8 DoubleRow matmuls (K=256 per instruction,
2x PE throughput vs fp16) accumulating in fp32 PSUM (t-on-partitions x
o-free tiles, contraction over 4 k-pair blocks), DVE bias add (in fp32 x128
domain, fp16 out), ACT Silu with scale=1/128 folding the weight scale back
out (fp16 out), fp16 store. x loads ride the SP HWDGE ring; weight loads and
output stores ride the ACT HWDGE ring.
"""

import ml_dtypes
import numpy as np

import concourse.bass as bass
import concourse.mybir as mybir
import concourse.tile as tile
from concourse.bass_utils import run_bass_kernel_spmd

N_CORES = 8
N_TOKENS = 65536
D_IN = 1024
D_OUT = 2048
N_EXPERTS = 16
E_PER_CORE = N_EXPERTS // N_CORES        # 2
TOK_PER_CORE = N_TOKENS // N_CORES       # 8192
TOK_PER_EXPERT = N_TOKENS // N_EXPERTS   # 4096

P = 128
KB = D_IN // P            # 8 contraction blocks
TS = 512                  # token stripe
OB = 512                  # out-feature block (one PSUM bank)
N_OB = D_OUT // OB        # 4
N_TSUB = TS // P          # 4
STRIPES_PER_EXPERT = TOK_PER_EXPERT // TS  # 8

F32 = mybir.dt.float32
F16 = mybir.dt.float16
F8 = mybir.dt.float8e4
NP_F8 = ml_dtypes.float8_e4m3
W_SCALE = 128.0  # lifts w_std ~0.0054 out of e4m3 subnormal territory
KH_G = 2          # kb per W tile = one DoubleRow k-pair
NWT_G = KB // KH_G  # 4 W tiles per expert


def _split_multi_waits(nc):
    """This container's walrus encodes at most ONE sync-wait per instruction;
    hoist extras onto single-wait NoOps inserted just before, same engine."""
    for fn in nc.m.functions:
        for bb in fn.blocks:
            insts = list(bb.instructions)
            out = []
            dirty = False
            for inst in insts:
                si = inst.sync_info
                waits = list(si.on_wait) if si and si.on_wait else []
                if len(waits) > 1:
                    dirty = True
                    for j, w in enumerate(waits[:-1]):
                        nop = mybir.InstNoOp(
                            name=f"{inst.name}-prewait{j}", ins=[], outs=[]
                        )
                        nop.engine = inst.engine
                        nop.sync_info = mybir.SyncInfo(on_wait=[w], on_update=[])
                        out.append(nop)
                    inst.sync_info = mybir.SyncInfo(
                        on_wait=[waits[-1]],
                        on_update=list(si.on_update) if si.on_update else [],
                    )
                out.append(inst)
            if dirty:
                bb.instructions = out


N_STRIPES = E_PER_CORE * STRIPES_PER_EXPERT  # 16


def build_kernel():
    nc = bass.Bass()
    # xt pre-striped on host: [stripe, partition, kb, t] so each stripe loads
    # with 8KB-contiguous per-partition lines
    xt = nc.dram_tensor("xt", [N_STRIPES, P, KB, TS], F8, kind="ExternalInput")
    # w pre-packed on host into the exact sbuf tile layout [e, h, p, os, kh, o']
    # so W DMAs are fully contiguous per-partition reads with 4KB (full-tile)
    # or 2KB (os-half) elements — the naive "(kb p) o" rearrange reads
    # scattered 1-2KB chunks at a fraction of the per-queue bandwidth
    w = nc.dram_tensor(
        "w", [E_PER_CORE, NWT_G, P, 2, KH_G, D_OUT // 2], F8, kind="ExternalInput"
    )
    # bias pre-broadcast on host in fp16 (512KB/expert; fp32 was 1MB of
    # redundant DMA sitting in front of ramp-critical W slices, and the
    # on-device partition_broadcast op doesn't encode in this toolchain)
    bb = nc.dram_tensor("bb", [E_PER_CORE, P, D_OUT], F16, kind="ExternalInput")
    y = nc.dram_tensor("y", [TOK_PER_CORE, D_OUT], F16, kind="ExternalOutput")

    with tile.TileContext(nc) as tc:
        with (
            tc.tile_pool(name="persist", bufs=1) as persist,
            tc.tile_pool(name="xp", bufs=3) as xp,
            tc.tile_pool(name="outs", bufs=3) as outs,
            tc.tile_pool(name="psum", bufs=8, space="PSUM") as psump,
        ):
            x16_tiles = {}
            x16_tiles[0] = xp.tile([P, KB, TS], F8, tag="x16", name="x16_s0")
            nc.sync.dma_start(x16_tiles[0][:], xt[0])

            KH = KH_G
            NWT = NWT_G
            b_sb = [
                persist.tile([P, D_OUT], F16, name=f"bias_{e}")
                for e in range(E_PER_CORE)
            ]
            # os-major W tiles: [p, os, kh, o'] so an os-half is a contiguous
            # 2KB run per partition (DMA elem size drives queue bandwidth)
            w16 = [
                [
                    persist.tile([P, 2, KH, D_OUT // 2], F8, name=f"w16_{e}_{h}")
                    for h in range(NWT)
                ]
                for e in range(E_PER_CORE)
            ]

            half = D_OUT // 2

            def load_expert0():
                # expert 0 gates the ramp: slice into need-ordered 256KB
                # os-halves across two rings so the first accumulation groups
                # unblock early; each queue tops out well under the core's
                # aggregate HBM rate, so splitting rings + large elements is
                # what shortens the stall. Bias halves slot in right after
                # the os0 W pieces (first DVE read comes ~2us after the
                # first matmul, with 8 PSUM banks of runway).
                for h in range(NWT):
                    eng = nc.scalar if h % 2 == 0 else nc.gpsimd
                    eng.dma_start(w16[0][h][:, 0], w[0, h, :, 0])
                nc.gpsimd.dma_start(b_sb[0][:, :half], bb[0][:, :half])
                nc.sync.dma_start(b_sb[0][:, half:], bb[0][:, half:])
                for h in range(NWT):
                    eng = nc.scalar if h % 2 == 0 else nc.gpsimd
                    eng.dma_start(w16[0][h][:, 1], w[0, h, :, 1])

            def load_expert1():
                # mid-flight, off the critical path: full-tile 512KB DMAs
                # with 4KB elements
                nc.gpsimd.dma_start(b_sb[1][:], bb[1])
                for h in range(NWT):
                    eng = nc.scalar if h % 2 == 0 else nc.sync
                    eng.dma_start(w16[1][h][:], w[1, h])

            load_expert0()

            # PE warmup: dummy DoubleRow matmuls on a memset scratch tile so
            # the tensor engine is at full p-state clock (not the 1.2GHz ramp
            # tier) by the time the critical preload lands; also converts the
            # ~5us data-starved head into busy time
            wu = persist.tile([P, 2, OB], F8, name="warmup")
            nc.vector.memset(wu[:], 0)
            wu_ps = psump.tile([P, OB], F32, tag="ps")
            N_WU = 10
            for i in range(N_WU):
                nc.tensor.matmul(
                    wu_ps[:],
                    lhsT=wu[:, :, 0:P],
                    rhs=wu[:],
                    start=(i == 0),
                    stop=(i == N_WU - 1),
                    perf_mode=mybir.MatmulPerfMode.DoubleRow,
                )

            def mm_group(x16, e, tsub, ob, ps):
                os_, oc = divmod(ob, 2)
                for h in range(NWT):
                    # DoubleRow: K=256 (one kb pair) per matmul
                    nc.tensor.matmul(
                        ps[:],
                        lhsT=x16[:, 2 * h:2 * h + 2, tsub * P:(tsub + 1) * P],
                        rhs=w16[e][h][:, os_, :, oc * OB:(oc + 1) * OB],
                        start=(h == 0),
                        stop=(h == NWT - 1),
                        perf_mode=mybir.MatmulPerfMode.DoubleRow,
                    )

            # stripe 0, ob-major: all os0 groups first so the PE ramp only
            # waits on the first half of expert-0's weights; per-half silu +
            # store keeps downstream engines streaming during the ramp
            for os_ in range(2):
                for tsub in range(N_TSUB):
                    yp = outs.tile([P, half], F16, tag="ypreh")
                    ya = outs.tile([P, half], F16, tag="yacth")
                    for oc in range(2):
                        ob = os_ * 2 + oc
                        ps = psump.tile([P, OB], F32, tag="ps")
                        mm_group(x16_tiles[0], 0, tsub, ob, ps)
                        nc.vector.tensor_tensor(
                            yp[:, oc * OB:(oc + 1) * OB], ps[:],
                            b_sb[0][:, ob * OB:(ob + 1) * OB],
                            mybir.AluOpType.add,
                        )
                    nc.scalar.activation(
                        ya[:], yp[:],
                        mybir.ActivationFunctionType.Silu,
                        scale=1.0 / W_SCALE,
                    )
                    [nc.gpsimd, nc.sync, nc.scalar][
                        (os_ * N_TSUB + tsub) % 3
                    ].dma_start(
                        y[tsub * P:(tsub + 1) * P,
                          os_ * half:(os_ + 1) * half],
                        ya[:],
                    )

            for e in range(E_PER_CORE):
                for s in range(STRIPES_PER_EXPERT):
                    g = e * STRIPES_PER_EXPERT + s
                    if g == 0:
                        continue  # handled above, ob-major
                    t0 = g * TS
                    x16 = xp.tile([P, KB, TS], F8, tag="x16", name="x16")
                    nc.sync.dma_start(x16[:], xt[g])

                    for tsub in range(N_TSUB):
                        last = g == N_STRIPES - 1 and tsub == N_TSUB - 1
                        store_eng = [nc.gpsimd, nc.sync, nc.scalar][
                            (g * N_TSUB + tsub) % 3
                        ]
                        y_pre = outs.tile([P, D_OUT], F16, tag="ypre")
                        y_act = outs.tile([P, D_OUT], F16, tag="yact")
                        for ob in range(N_OB):
                            ps = psump.tile([P, OB], F32, tag="ps")
                            mm_group(x16, e, tsub, ob, ps)
                            # bias add in the x128 domain (bias pre-scaled on
                            # host); fp16 out is exact enough at |v|<~700
                            nc.vector.tensor_tensor(
                                y_pre[:, ob * OB:(ob + 1) * OB], ps[:],
                                b_sb[e][:, ob * OB:(ob + 1) * OB],
                                mybir.AluOpType.add,
                            )
                            if last:
                                # final tile: silu+store per-ob so the tail
                                # chain after the last matmul stays short
                                nc.scalar.activation(
                                    y_act[:, ob * OB:(ob + 1) * OB],
                                    y_pre[:, ob * OB:(ob + 1) * OB],
                                    mybir.ActivationFunctionType.Silu,
                                    scale=1.0 / W_SCALE,
                                )
                                [nc.gpsimd, nc.sync, nc.scalar][ob % 3].dma_start(
                                    y[t0 + tsub * P:t0 + (tsub + 1) * P,
                                      ob * OB:(ob + 1) * OB],
                                    y_act[:, ob * OB:(ob + 1) * OB],
                                )
                        if not last:
                            # one fused silu per 2048-wide tile amortizes the
                            # ~300ns fixed ACT cost; scale folds the x128
                            # weight scale back out before the nonlinearity
                            nc.scalar.activation(
                                y_act[:], y_pre[:],
                                mybir.ActivationFunctionType.Silu,
                                scale=1.0 / W_SCALE,
                            )
                            store_eng.dma_start(
                                y[t0 + tsub * P:t0 + (tsub + 1) * P, :], y_act[:]
                            )
                    if g == 1:
                        # after g==1 so expert 1's 1MB on the sync ring sits
                        # behind the already-enqueued x1/x2 prefetches
                        load_expert1()

    _split_multi_waits(nc)
    return nc


_NC_CACHE = None


def _get_nc():
    global _NC_CACHE
    if _NC_CACHE is None:
        _NC_CACHE = build_kernel()
    return _NC_CACHE


def _in_maps(sorted_features, routing_matrix, routing_bias):
    maps = []
    for c in range(N_CORES):
        rows = slice(c * TOK_PER_CORE, (c + 1) * TOK_PER_CORE)
        es = slice(c * E_PER_CORE, (c + 1) * E_PER_CORE)
        # [stripe, partition, kb, t]: element (s,p,kb,t) = X_c[s*TS+t, kb*P+p]
        xt_c = np.ascontiguousarray(
            sorted_features[rows]
            .reshape(N_STRIPES, TS, KB, P)
            .transpose(0, 3, 2, 1)
            .astype(NP_F8)
        )
        # pack into the device tile layout [e, h, p, os, kh, o'] so each W
        # DMA reads fully contiguous blocks: kin = (h*KH+kh)*128 + p
        w_c = np.ascontiguousarray(
            (routing_matrix[:, :, es].transpose(2, 0, 1) * W_SCALE)
            .astype(NP_F8)
            .reshape(E_PER_CORE, NWT_G, KH_G, P, 2, D_OUT // 2)
            .transpose(0, 1, 3, 4, 2, 5)
        )
        # bias enters the DVE add in the x128 domain: silu((ps + S*b)/S);
        # fp16 is exact to ~2^-11 relative, far under the fp8 matmul noise
        b_c = np.ascontiguousarray(
            np.broadcast_to(
                (routing_bias[:, es].T * W_SCALE)[:, None, :],
                (E_PER_CORE, P, D_OUT),
            ).astype(np.float16)
        )
        maps.append({"xt": xt_c, "w": w_c, "bb": b_c})
    return maps


def run(sorted_features, routing_matrix, routing_bias, **run_kwargs):
    nc = _get_nc()
    maps = _in_maps(sorted_features, routing_matrix, routing_bias)
    res = run_bass_kernel_spmd(nc, maps, core_ids=list(range(N_CORES)), **run_kwargs)
    out = np.concatenate(
        [res.results[c]["y"].astype(np.float32) for c in range(N_CORES)], axis=0
    )
    return out, res


def kernel(sorted_features, expert_ids_sorted, routing_matrix, routing_bias):
    assert sorted_features.shape == (N_TOKENS, D_IN)
    assert routing_matrix.shape == (D_IN, D_OUT, N_EXPERTS)
    assert routing_bias.shape == (D_OUT, N_EXPERTS)
    out, _ = run(
        np.asarray(sorted_features, dtype=np.float32),
        np.asarray(routing_matrix, dtype=np.float32),
        np.asarray(routing_bias, dtype=np.float32),
    )
    return out



# revision 43
# speedup vs baseline: 1.0590x; 1.0008x over previous
"""MoE expert-collection grouped GEMM for Trainium2, expert-parallel over 8
NeuronCores.

Problem (hardcoded shapes):
  sorted_features  [65536, 1024] f32   tokens sorted by expert, 4096/expert
  expert_ids_sorted[65536] i32         unused: split is static equal-count
  routing_matrix   [1024, 2048, 16] f32
  routing_bias     [2048, 16] f32
  out = silu(x_e @ W_e + b_e) per expert  -> [65536, 2048] f32

Sharding: expert-parallel, 2 experts (= 8192 contiguous sorted tokens) per
core. Host-side dispatch hands each core its token block transposed
(feature-major, fp8 e4m3) plus its 2 experts' weights (fp8 e4m3, pre-scaled
x128 so w_std 0.0054 lands in e4m3's normal range) and bias pre-broadcast to
128 partitions (fp32, pre-scaled x128 to match).

Device pipeline per core: 1024 fp8 DoubleRow matmuls (K=256 per instruction,
2x PE throughput vs fp16) accumulating in fp32 PSUM (t-on-partitions x
o-free tiles, contraction over 4 k-pair blocks), DVE bias add (in fp32 x128
domain, fp16 out), ACT Silu with scale=1/128 folding the weight scale back
out (fp16 out), fp16 store. x loads ride the SP HWDGE ring; weight loads and
output stores ride the ACT HWDGE ring.
"""

import ml_dtypes
import numpy as np

import concourse.bass as bass
import concourse.mybir as mybir
import concourse.tile as tile
from concourse.bass_utils import run_bass_kernel_spmd

N_CORES = 8
N_TOKENS = 65536
D_IN = 1024
D_OUT = 2048
N_EXPERTS = 16
E_PER_CORE = N_EXPERTS // N_CORES        # 2
TOK_PER_CORE = N_TOKENS // N_CORES       # 8192
TOK_PER_EXPERT = N_TOKENS // N_EXPERTS   # 4096

P = 128
KB = D_IN // P            # 8 contraction blocks
TS = 512                  # token stripe
OB = 512                  # out-feature block (one PSUM bank)
N_OB = D_OUT // OB        # 4
N_TSUB = TS // P          # 4
STRIPES_PER_EXPERT = TOK_PER_EXPERT // TS  # 8

F32 = mybir.dt.float32
F16 = mybir.dt.float16
F8 = mybir.dt.float8e4
NP_F8 = ml_dtypes.float8_e4m3
W_SCALE = 128.0  # lifts w_std ~0.0054 out of e4m3 subnormal territory
KH_G = 2          # kb per W tile = one DoubleRow k-pair
NWT_G = KB // KH_G  # 4 W tiles per expert


def _split_multi_waits(nc):
    """This container's walrus encodes at most ONE sync-wait per instruction;
    hoist extras onto single-wait NoOps inserted just before, same engine."""
    for fn in nc.m.functions:
        for bb in fn.blocks:
            insts = list(bb.instructions)
            out = []
            dirty = False
            for inst in insts:
                si = inst.sync_info
                waits = list(si.on_wait) if si and si.on_wait else []
                if len(waits) > 1:
                    dirty = True
                    for j, w in enumerate(waits[:-1]):
                        nop = mybir.InstNoOp(
                            name=f"{inst.name}-prewait{j}", ins=[], outs=[]
                        )
                        nop.engine = inst.engine
                        nop.sync_info = mybir.SyncInfo(on_wait=[w], on_update=[])
                        out.append(nop)
                    inst.sync_info = mybir.SyncInfo(
                        on_wait=[waits[-1]],
                        on_update=list(si.on_update) if si.on_update else [],
                    )
                out.append(inst)
            if dirty:
                bb.instructions = out


N_STRIPES = E_PER_CORE * STRIPES_PER_EXPERT  # 16


def build_kernel():
    nc = bass.Bass()
    # xt pre-striped on host: [stripe, partition, kb, t] so each stripe loads
    # with 8KB-contiguous per-partition lines
    xt = nc.dram_tensor("xt", [N_STRIPES, P, KB, TS], F8, kind="ExternalInput")
    # w pre-packed on host into the exact sbuf tile layout [e, h, p, os, kh, o']
    # so W DMAs are fully contiguous per-partition reads with 4KB (full-tile)
    # or 2KB (os-half) elements — the naive "(kb p) o" rearrange reads
    # scattered 1-2KB chunks at a fraction of the per-queue bandwidth
    w = nc.dram_tensor(
        "w", [E_PER_CORE, NWT_G, P, 2, 2, KH_G, D_OUT // 4], F8,
        kind="ExternalInput",
    )
    # bias pre-broadcast on host in fp16 (512KB/expert; fp32 was 1MB of
    # redundant DMA sitting in front of ramp-critical W slices, and the
    # on-device partition_broadcast op doesn't encode in this toolchain)
    bb = nc.dram_tensor("bb", [E_PER_CORE, P, D_OUT], F16, kind="ExternalInput")
    y = nc.dram_tensor("y", [TOK_PER_CORE, D_OUT], F16, kind="ExternalOutput")

    with tile.TileContext(nc) as tc:
        with (
            tc.tile_pool(name="persist", bufs=1) as persist,
            tc.tile_pool(name="xp", bufs=3) as xp,
            tc.tile_pool(name="outs", bufs=3) as outs,
            tc.tile_pool(name="psum", bufs=8, space="PSUM") as psump,
        ):
            x16_tiles = {}
            x16_tiles[0] = xp.tile([P, KB, TS], F8, tag="x16", name="x16_s0")
            nc.sync.dma_start(x16_tiles[0][:], xt[0])

            KH = KH_G
            NWT = NWT_G
            b_sb = [
                persist.tile([P, D_OUT], F16, name=f"bias_{e}")
                for e in range(E_PER_CORE)
            ]
            # block-major W tiles: [p, os, oc, kh, o''] so os-halves (2KB) and
            # oc-quarters (1KB) are contiguous per-partition runs — DMA slices
            # at any preload granularity keep large elements
            w16 = [
                [
                    persist.tile(
                        [P, 2, 2, KH, D_OUT // 4], F8, name=f"w16_{e}_{h}"
                    )
                    for h in range(NWT)
                ]
                for e in range(E_PER_CORE)
            ]

            half = D_OUT // 2

            def load_expert0():
                # expert 0 gates the ramp. Cold queues only deliver
                # ~55GB/s each, so the first full accumulation group (which
                # needs an oc-column of ALL FOUR h tiles) is bounded by the
                # 2-deep piece on its queue: use 128KB oc-quarters for the
                # os0 half so that depth costs ~2.3us, not ~4.6us. Bias
                # first-half rides sync behind x0 (first DVE read comes
                # ~2us after the first matmul, 8 PSUM banks of runway).
                for oc in range(2):
                    for h in range(NWT):
                        eng = nc.scalar if h % 2 == 0 else nc.gpsimd
                        eng.dma_start(
                            w16[0][h][:, 0, oc], w[0, h, :, 0, oc]
                        )
                nc.sync.dma_start(b_sb[0][:, :half], bb[0][:, :half])
                for h in range(NWT):
                    eng = nc.scalar if h % 2 == 0 else nc.gpsimd
                    eng.dma_start(w16[0][h][:, 1], w[0, h, :, 1])
                nc.gpsimd.dma_start(b_sb[0][:, half:], bb[0][:, half:])

            def load_expert1():
                # mid-flight, off the critical path: full-tile 512KB DMAs
                # with 4KB elements
                nc.gpsimd.dma_start(b_sb[1][:], bb[1])
                for h in range(NWT):
                    eng = nc.scalar if h % 2 == 0 else nc.sync
                    eng.dma_start(w16[1][h][:], w[1, h])

            load_expert0()

            # PE warmup: dummy DoubleRow matmuls on a memset scratch tile so
            # the tensor engine is at full p-state clock (not the 1.2GHz ramp
            # tier) by the time the critical preload lands; also converts the
            # ~5us data-starved head into busy time
            wu = persist.tile([P, 2, OB], F8, name="warmup")
            nc.vector.memset(wu[:], 0)
            wu_ps = psump.tile([P, OB], F32, tag="ps")
            N_WU = 10
            for i in range(N_WU):
                nc.tensor.matmul(
                    wu_ps[:],
                    lhsT=wu[:, :, 0:P],
                    rhs=wu[:],
                    start=(i == 0),
                    stop=(i == N_WU - 1),
                    perf_mode=mybir.MatmulPerfMode.DoubleRow,
                )

            def mm_group(x16, e, tsub, ob, ps):
                os_, oc = divmod(ob, 2)
                for h in range(NWT):
                    # DoubleRow: K=256 (one kb pair) per matmul
                    nc.tensor.matmul(
                        ps[:],
                        lhsT=x16[:, 2 * h:2 * h + 2, tsub * P:(tsub + 1) * P],
                        rhs=w16[e][h][:, os_, oc],
                        start=(h == 0),
                        stop=(h == NWT - 1),
                        perf_mode=mybir.MatmulPerfMode.DoubleRow,
                    )

            # stripe 0, ob-major: all os0 groups first so the PE ramp only
            # waits on the first half of expert-0's weights; per-half silu +
            # store keeps downstream engines streaming during the ramp
            for os_ in range(2):
                for tsub in range(N_TSUB):
                    yp = outs.tile([P, half], F16, tag="ypreh")
                    ya = outs.tile([P, half], F16, tag="yacth")
                    for oc in range(2):
                        ob = os_ * 2 + oc
                        ps = psump.tile([P, OB], F32, tag="ps")
                        mm_group(x16_tiles[0], 0, tsub, ob, ps)
                        nc.vector.tensor_tensor(
                            yp[:, oc * OB:(oc + 1) * OB], ps[:],
                            b_sb[0][:, ob * OB:(ob + 1) * OB],
                            mybir.AluOpType.add,
                        )
                    nc.scalar.activation(
                        ya[:], yp[:],
                        mybir.ActivationFunctionType.Silu,
                        scale=1.0 / W_SCALE,
                    )
                    [nc.gpsimd, nc.sync, nc.scalar][
                        (os_ * N_TSUB + tsub) % 3
                    ].dma_start(
                        y[tsub * P:(tsub + 1) * P,
                          os_ * half:(os_ + 1) * half],
                        ya[:],
                    )

            for e in range(E_PER_CORE):
                for s in range(STRIPES_PER_EXPERT):
                    g = e * STRIPES_PER_EXPERT + s
                    if g == 0:
                        continue  # handled above, ob-major
                    t0 = g * TS
                    x16 = xp.tile([P, KB, TS], F8, tag="x16", name="x16")
                    nc.sync.dma_start(x16[:], xt[g])

                    if g == N_STRIPES - 1:
                        # final stripe: per os-half silu + 256KB stores,
                        # spread over all rings, so the post-matmul chain and
                        # queue backlogs drain fast
                        for tsub in range(N_TSUB):
                            for os_ in range(2):
                                yp = outs.tile([P, half], F16, tag="ypreh")
                                ya = outs.tile([P, half], F16, tag="yacth")
                                for oc in range(2):
                                    ob = os_ * 2 + oc
                                    ps = psump.tile([P, OB], F32, tag="ps")
                                    mm_group(x16, e, tsub, ob, ps)
                                    nc.vector.tensor_tensor(
                                        yp[:, oc * OB:(oc + 1) * OB], ps[:],
                                        b_sb[e][:, ob * OB:(ob + 1) * OB],
                                        mybir.AluOpType.add,
                                    )
                                nc.scalar.activation(
                                    ya[:], yp[:],
                                    mybir.ActivationFunctionType.Silu,
                                    scale=1.0 / W_SCALE,
                                )
                                [nc.gpsimd, nc.sync, nc.scalar][
                                    (tsub * 2 + os_) % 3
                                ].dma_start(
                                    y[t0 + tsub * P:t0 + (tsub + 1) * P,
                                      os_ * half:(os_ + 1) * half],
                                    ya[:],
                                )
                        continue

                    for tsub in range(N_TSUB):
                        store_eng = [nc.gpsimd, nc.sync, nc.scalar][
                            (g * N_TSUB + tsub) % 3
                        ]
                        y_pre = outs.tile([P, D_OUT], F16, tag="ypre")
                        y_act = outs.tile([P, D_OUT], F16, tag="yact")
                        for ob in range(N_OB):
                            ps = psump.tile([P, OB], F32, tag="ps")
                            mm_group(x16, e, tsub, ob, ps)
                            # bias add in the x128 domain (bias pre-scaled on
                            # host); fp16 out is exact enough at |v|<~700
                            nc.vector.tensor_tensor(
                                y_pre[:, ob * OB:(ob + 1) * OB], ps[:],
                                b_sb[e][:, ob * OB:(ob + 1) * OB],
                                mybir.AluOpType.add,
                            )
                        # one fused silu per 2048-wide tile amortizes the
                        # ~300ns fixed ACT cost; scale folds the x128
                        # weight scale back out before the nonlinearity
                        nc.scalar.activation(
                            y_act[:], y_pre[:],
                            mybir.ActivationFunctionType.Silu,
                            scale=1.0 / W_SCALE,
                        )
                        store_eng.dma_start(
                            y[t0 + tsub * P:t0 + (tsub + 1) * P, :], y_act[:]
                        )
                    if g == 1:
                        # after g==1 so expert 1's 1MB on the sync ring sits
                        # behind the already-enqueued x1/x2 prefetches
                        load_expert1()

    _split_multi_waits(nc)
    return nc


_NC_CACHE = None


def _get_nc():
    global _NC_CACHE
    if _NC_CACHE is None:
        _NC_CACHE = build_kernel()
    return _NC_CACHE


def _in_maps(sorted_features, routing_matrix, routing_bias):
    maps = []
    for c in range(N_CORES):
        rows = slice(c * TOK_PER_CORE, (c + 1) * TOK_PER_CORE)
        es = slice(c * E_PER_CORE, (c + 1) * E_PER_CORE)
        # [stripe, partition, kb, t]: element (s,p,kb,t) = X_c[s*TS+t, kb*P+p]
        xt_c = np.ascontiguousarray(
            sorted_features[rows]
            .reshape(N_STRIPES, TS, KB, P)
            .transpose(0, 3, 2, 1)
            .astype(NP_F8)
        )
        # pack into the device tile layout [e, h, p, os, oc, kh, o''] so each
        # W DMA reads fully contiguous blocks: kin = (h*KH+kh)*128 + p,
        # o = os*1024 + oc*512 + o''
        w_c = np.ascontiguousarray(
            (routing_matrix[:, :, es].transpose(2, 0, 1) * W_SCALE)
            .astype(NP_F8)
            .reshape(E_PER_CORE, NWT_G, KH_G, P, 2, 2, D_OUT // 4)
            .transpose(0, 1, 3, 4, 5, 2, 6)
        )
        # bias enters the DVE add in the x128 domain: silu((ps + S*b)/S);
        # fp16 is exact to ~2^-11 relative, far under the fp8 matmul noise
        b_c = np.ascontiguousarray(
            np.broadcast_to(
                (routing_bias[:, es].T * W_SCALE)[:, None, :],
                (E_PER_CORE, P, D_OUT),
            ).astype(np.float16)
        )
        maps.append({"xt": xt_c, "w": w_c, "bb": b_c})
    return maps


def run(sorted_features, routing_matrix, routing_bias, **run_kwargs):
    nc = _get_nc()
    maps = _in_maps(sorted_features, routing_matrix, routing_bias)
    res = run_bass_kernel_spmd(nc, maps, core_ids=list(range(N_CORES)), **run_kwargs)
    out = np.concatenate(
        [res.results[c]["y"].astype(np.float32) for c in range(N_CORES)], axis=0
    )
    return out, res


def kernel(sorted_features, expert_ids_sorted, routing_matrix, routing_bias):
    assert sorted_features.shape == (N_TOKENS, D_IN)
    assert routing_matrix.shape == (D_IN, D_OUT, N_EXPERTS)
    assert routing_bias.shape == (D_OUT, N_EXPERTS)
    out, _ = run(
        np.asarray(sorted_features, dtype=np.float32),
        np.asarray(routing_matrix, dtype=np.float32),
        np.asarray(routing_bias, dtype=np.float32),
    )
    return out



# revision 49
# speedup vs baseline: 1.0633x; 1.0041x over previous
"""MoE expert-collection grouped GEMM for Trainium2, expert-parallel over 8
NeuronCores.

Problem (hardcoded shapes):
  sorted_features  [65536, 1024] f32   tokens sorted by expert, 4096/expert
  expert_ids_sorted[65536] i32         unused: split is static equal-count
  routing_matrix   [1024, 2048, 16] f32
  routing_bias     [2048, 16] f32
  out = silu(x_e @ W_e + b_e) per expert  -> [65536, 2048] f32

Sharding: expert-parallel, 2 experts (= 8192 contiguous sorted tokens) per
core. Host-side dispatch hands each core its token block transposed
(feature-major, fp8 e4m3) plus its 2 experts' weights (fp8 e4m3, pre-scaled
x128 so w_std 0.0054 lands in e4m3's normal range) and bias pre-broadcast to
128 partitions (fp32, pre-scaled x128 to match).

Device pipeline per core: 1024 fp8 DoubleRow matmuls (K=256 per instruction,
2x PE throughput vs fp16) accumulating in fp32 PSUM (t-on-partitions x
o-free tiles, contraction over 4 k-pair blocks), DVE bias add (in fp32 x128
domain, fp16 out), ACT Silu with scale=1/128 folding the weight scale back
out (fp16 out), fp16 store. x loads ride the SP HWDGE ring; weight loads and
output stores ride the ACT HWDGE ring.
"""

import ml_dtypes
import numpy as np

import concourse.bass as bass
import concourse.mybir as mybir
import concourse.tile as tile
from concourse.bass_utils import run_bass_kernel_spmd

N_CORES = 8
N_TOKENS = 65536
D_IN = 1024
D_OUT = 2048
N_EXPERTS = 16
E_PER_CORE = N_EXPERTS // N_CORES        # 2
TOK_PER_CORE = N_TOKENS // N_CORES       # 8192
TOK_PER_EXPERT = N_TOKENS // N_EXPERTS   # 4096

P = 128
KB = D_IN // P            # 8 contraction blocks
TS = 512                  # token stripe
OB = 512                  # out-feature block (one PSUM bank)
N_OB = D_OUT // OB        # 4
N_TSUB = TS // P          # 4
STRIPES_PER_EXPERT = TOK_PER_EXPERT // TS  # 8

F32 = mybir.dt.float32
F16 = mybir.dt.float16
F8 = mybir.dt.float8e4
NP_F8 = ml_dtypes.float8_e4m3
W_SCALE = 128.0  # lifts w_std ~0.0054 out of e4m3 subnormal territory
KH_G = 2          # kb per W tile = one DoubleRow k-pair
NWT_G = KB // KH_G  # 4 W tiles per expert


def _split_multi_waits(nc):
    """This container's walrus encodes at most ONE sync-wait per instruction;
    hoist extras onto single-wait NoOps inserted just before, same engine."""
    for fn in nc.m.functions:
        for bb in fn.blocks:
            insts = list(bb.instructions)
            out = []
            dirty = False
            for inst in insts:
                si = inst.sync_info
                waits = list(si.on_wait) if si and si.on_wait else []
                if len(waits) > 1:
                    dirty = True
                    for j, w in enumerate(waits[:-1]):
                        nop = mybir.InstNoOp(
                            name=f"{inst.name}-prewait{j}", ins=[], outs=[]
                        )
                        nop.engine = inst.engine
                        nop.sync_info = mybir.SyncInfo(on_wait=[w], on_update=[])
                        out.append(nop)
                    inst.sync_info = mybir.SyncInfo(
                        on_wait=[waits[-1]],
                        on_update=list(si.on_update) if si.on_update else [],
                    )
                out.append(inst)
            if dirty:
                bb.instructions = out


N_STRIPES = E_PER_CORE * STRIPES_PER_EXPERT  # 16


def build_kernel():
    nc = bass.Bass()
    # xt pre-striped on host: [stripe, partition, kb, t] so each stripe loads
    # with 8KB-contiguous per-partition lines
    xt = nc.dram_tensor("xt", [N_STRIPES, P, KB, TS], F8, kind="ExternalInput")
    # w pre-packed on host into the exact sbuf tile layout [e, h, p, os, kh, o']
    # so W DMAs are fully contiguous per-partition reads with 4KB (full-tile)
    # or 2KB (os-half) elements — the naive "(kb p) o" rearrange reads
    # scattered 1-2KB chunks at a fraction of the per-queue bandwidth
    # per (expert, queue) pack: q carries h∈{q, q+2}; one 512KB DMA delivers
    # a full os-half for both h tiles (cold transfers cost ~4-5us nearly
    # independent of size, so the ramp wants FEW, LARGE transfers)
    w = nc.dram_tensor(
        "w", [E_PER_CORE, 2, P, 2, 2, 2, KH_G, D_OUT // 4], F8,
        kind="ExternalInput",
    )
    # bias pre-broadcast on host in fp16 (512KB/expert; fp32 was 1MB of
    # redundant DMA sitting in front of ramp-critical W slices, and the
    # on-device partition_broadcast op doesn't encode in this toolchain)
    bb = nc.dram_tensor("bb", [E_PER_CORE, P, D_OUT], F16, kind="ExternalInput")
    y = nc.dram_tensor("y", [TOK_PER_CORE, D_OUT], F16, kind="ExternalOutput")

    with tile.TileContext(nc) as tc:
        with (
            tc.tile_pool(name="persist", bufs=1) as persist,
            tc.tile_pool(name="xp", bufs=3) as xp,
            tc.tile_pool(name="outs", bufs=3) as outs,
            tc.tile_pool(name="psum", bufs=8, space="PSUM") as psump,
        ):
            x16_tiles = {}
            x16_tiles[0] = xp.tile([P, KB, TS], F8, tag="x16", name="x16_s0")
            nc.sync.dma_start(x16_tiles[0][:], xt[0])

            KH = KH_G
            NWT = NWT_G
            b_sb = [
                persist.tile([P, D_OUT], F16, name=f"bias_{e}")
                for e in range(E_PER_CORE)
            ]
            # merged per-queue W tiles: [p, os, h', oc, kh, o''] where queue q
            # holds h ∈ {q, q+2}; an os-half of a whole queue is one
            # contiguous 4KB-per-partition run = one large DMA
            w16 = [
                [
                    persist.tile(
                        [P, 2, 2, 2, KH, D_OUT // 4], F8, name=f"w16_{e}_q{q}"
                    )
                    for q in range(2)
                ]
                for e in range(E_PER_CORE)
            ]

            half = D_OUT // 2

            W_ENG = [nc.scalar, nc.gpsimd]

            def load_expert0():
                # expert 0 gates the ramp, and cold transfers cost ~4-5us
                # each nearly independent of size — so deliver each queue's
                # whole os0 (512KB, needed by the first 8 groups) as ONE
                # transfer, then os1. Bias first-half rides sync behind x0
                # (first DVE read comes ~2us after the first matmul, with 8
                # PSUM banks of runway).
                for os_ in range(2):
                    for q in range(2):
                        W_ENG[q].dma_start(w16[0][q][:, os_], w[0, q, :, os_])
                    if os_ == 0:
                        nc.sync.dma_start(b_sb[0][:, :half], bb[0][:, :half])
                nc.gpsimd.dma_start(b_sb[0][:, half:], bb[0][:, half:])

            def load_expert1():
                # mid-flight, off the critical path: one warm 1MB transfer
                # per queue
                nc.gpsimd.dma_start(b_sb[1][:], bb[1])
                nc.scalar.dma_start(w16[1][0][:], w[1, 0])
                nc.sync.dma_start(w16[1][1][:], w[1, 1])

            load_expert0()

            # PE warmup: dummy DoubleRow matmuls on a memset scratch tile so
            # the tensor engine is at full p-state clock (not the 1.2GHz ramp
            # tier) by the time the critical preload lands; also converts the
            # ~5us data-starved head into busy time
            wu = persist.tile([P, 2, OB], F8, name="warmup")
            nc.vector.memset(wu[:], 0)
            wu_ps = psump.tile([P, OB], F32, tag="ps")
            N_WU = 13
            for i in range(N_WU):
                nc.tensor.matmul(
                    wu_ps[:],
                    lhsT=wu[:, :, 0:P],
                    rhs=wu[:],
                    start=(i == 0),
                    stop=(i == N_WU - 1),
                    perf_mode=mybir.MatmulPerfMode.DoubleRow,
                )

            def mm_group(x16, e, tsub, ob, ps):
                os_, oc = divmod(ob, 2)
                for h in range(NWT):
                    # DoubleRow: K=256 (one kb pair) per matmul; queue q=h%2
                    # holds h at slot h'=h//2
                    nc.tensor.matmul(
                        ps[:],
                        lhsT=x16[:, 2 * h:2 * h + 2, tsub * P:(tsub + 1) * P],
                        rhs=w16[e][h % 2][:, os_, h // 2, oc],
                        start=(h == 0),
                        stop=(h == NWT - 1),
                        perf_mode=mybir.MatmulPerfMode.DoubleRow,
                    )

            # stripe 0, ob-major: all os0 groups first so the PE ramp only
            # waits on the first half of expert-0's weights; per-half silu +
            # store keeps downstream engines streaming during the ramp
            for os_ in range(2):
                for tsub in range(N_TSUB):
                    yp = outs.tile([P, half], F16, tag="ypreh")
                    ya = outs.tile([P, half], F16, tag="yacth")
                    for oc in range(2):
                        ob = os_ * 2 + oc
                        ps = psump.tile([P, OB], F32, tag="ps")
                        mm_group(x16_tiles[0], 0, tsub, ob, ps)
                        nc.vector.tensor_tensor(
                            yp[:, oc * OB:(oc + 1) * OB], ps[:],
                            b_sb[0][:, ob * OB:(ob + 1) * OB],
                            mybir.AluOpType.add,
                        )
                    nc.scalar.activation(
                        ya[:], yp[:],
                        mybir.ActivationFunctionType.Silu,
                        scale=1.0 / W_SCALE,
                    )
                    [nc.gpsimd, nc.sync, nc.scalar][
                        (os_ * N_TSUB + tsub) % 3
                    ].dma_start(
                        y[tsub * P:(tsub + 1) * P,
                          os_ * half:(os_ + 1) * half],
                        ya[:],
                    )

            for e in range(E_PER_CORE):
                for s in range(STRIPES_PER_EXPERT):
                    g = e * STRIPES_PER_EXPERT + s
                    if g == 0:
                        continue  # handled above, ob-major
                    t0 = g * TS
                    x16 = xp.tile([P, KB, TS], F8, tag="x16", name="x16")
                    nc.sync.dma_start(x16[:], xt[g])

                    if g == N_STRIPES - 1:
                        # final stripe: per os-half silu + 256KB stores,
                        # spread over all rings, so the post-matmul chain and
                        # queue backlogs drain fast
                        for tsub in range(N_TSUB):
                            for os_ in range(2):
                                yp = outs.tile([P, half], F16, tag="ypreh")
                                ya = outs.tile([P, half], F16, tag="yacth")
                                for oc in range(2):
                                    ob = os_ * 2 + oc
                                    ps = psump.tile([P, OB], F32, tag="ps")
                                    mm_group(x16, e, tsub, ob, ps)
                                    nc.vector.tensor_tensor(
                                        yp[:, oc * OB:(oc + 1) * OB], ps[:],
                                        b_sb[e][:, ob * OB:(ob + 1) * OB],
                                        mybir.AluOpType.add,
                                    )
                                nc.scalar.activation(
                                    ya[:], yp[:],
                                    mybir.ActivationFunctionType.Silu,
                                    scale=1.0 / W_SCALE,
                                )
                                [nc.gpsimd, nc.sync, nc.scalar][
                                    (tsub * 2 + os_) % 3
                                ].dma_start(
                                    y[t0 + tsub * P:t0 + (tsub + 1) * P,
                                      os_ * half:(os_ + 1) * half],
                                    ya[:],
                                )
                        continue

                    for tsub in range(N_TSUB):
                        store_eng = [nc.gpsimd, nc.sync, nc.scalar][
                            (g * N_TSUB + tsub) % 3
                        ]
                        y_pre = outs.tile([P, D_OUT], F16, tag="ypre")
                        y_act = outs.tile([P, D_OUT], F16, tag="yact")
                        for ob in range(N_OB):
                            ps = psump.tile([P, OB], F32, tag="ps")
                            mm_group(x16, e, tsub, ob, ps)
                            # bias add in the x128 domain (bias pre-scaled on
                            # host); fp16 out is exact enough at |v|<~700
                            nc.vector.tensor_tensor(
                                y_pre[:, ob * OB:(ob + 1) * OB], ps[:],
                                b_sb[e][:, ob * OB:(ob + 1) * OB],
                                mybir.AluOpType.add,
                            )
                        # one fused silu per 2048-wide tile amortizes the
                        # ~300ns fixed ACT cost; scale folds the x128
                        # weight scale back out before the nonlinearity
                        nc.scalar.activation(
                            y_act[:], y_pre[:],
                            mybir.ActivationFunctionType.Silu,
                            scale=1.0 / W_SCALE,
                        )
                        store_eng.dma_start(
                            y[t0 + tsub * P:t0 + (tsub + 1) * P, :], y_act[:]
                        )
                    if g == 1:
                        # after g==1 so expert 1's 1MB on the sync ring sits
                        # behind the already-enqueued x1/x2 prefetches
                        load_expert1()

    _split_multi_waits(nc)
    return nc


_NC_CACHE = None


def _get_nc():
    global _NC_CACHE
    if _NC_CACHE is None:
        _NC_CACHE = build_kernel()
    return _NC_CACHE


def _in_maps(sorted_features, routing_matrix, routing_bias):
    maps = []
    for c in range(N_CORES):
        rows = slice(c * TOK_PER_CORE, (c + 1) * TOK_PER_CORE)
        es = slice(c * E_PER_CORE, (c + 1) * E_PER_CORE)
        # [stripe, partition, kb, t]: element (s,p,kb,t) = X_c[s*TS+t, kb*P+p]
        xt_c = np.ascontiguousarray(
            sorted_features[rows]
            .reshape(N_STRIPES, TS, KB, P)
            .transpose(0, 3, 2, 1)
            .astype(NP_F8)
        )
        # pack into the device tile layout [e, q, p, os, h', oc, kh, o'']:
        # kin = ((2h'+q)*KH+kh)*128 + p, o = os*1024 + oc*512 + o''
        w_c = np.ascontiguousarray(
            (routing_matrix[:, :, es].transpose(2, 0, 1) * W_SCALE)
            .astype(NP_F8)
            .reshape(E_PER_CORE, 2, 2, KH_G, P, 2, 2, D_OUT // 4)
            .transpose(0, 2, 4, 5, 1, 6, 3, 7)
        )
        # bias enters the DVE add in the x128 domain: silu((ps + S*b)/S);
        # fp16 is exact to ~2^-11 relative, far under the fp8 matmul noise
        b_c = np.ascontiguousarray(
            np.broadcast_to(
                (routing_bias[:, es].T * W_SCALE)[:, None, :],
                (E_PER_CORE, P, D_OUT),
            ).astype(np.float16)
        )
        maps.append({"xt": xt_c, "w": w_c, "bb": b_c})
    return maps


def run(sorted_features, routing_matrix, routing_bias, **run_kwargs):
    nc = _get_nc()
    maps = _in_maps(sorted_features, routing_matrix, routing_bias)
    res = run_bass_kernel_spmd(nc, maps, core_ids=list(range(N_CORES)), **run_kwargs)
    out = np.concatenate(
        [res.results[c]["y"].astype(np.float32) for c in range(N_CORES)], axis=0
    )
    return out, res


def kernel(sorted_features, expert_ids_sorted, routing_matrix, routing_bias):
    assert sorted_features.shape == (N_TOKENS, D_IN)
    assert routing_matrix.shape == (D_IN, D_OUT, N_EXPERTS)
    assert routing_bias.shape == (D_OUT, N_EXPERTS)
    out, _ = run(
        np.asarray(sorted_features, dtype=np.float32),
        np.asarray(routing_matrix, dtype=np.float32),
        np.asarray(routing_bias, dtype=np.float32),
    )
    return out



# revision 57
# speedup vs baseline: 1.0886x; 1.0238x over previous
"""MoE expert-collection grouped GEMM for Trainium2, expert-parallel over 8
NeuronCores.

Problem (hardcoded shapes):
  sorted_features  [65536, 1024] f32   tokens sorted by expert, 4096/expert
  expert_ids_sorted[65536] i32         unused: split is static equal-count
  routing_matrix   [1024, 2048, 16] f32
  routing_bias     [2048, 16] f32
  out = silu(x_e @ W_e + b_e) per expert  -> [65536, 2048] f32

Sharding: expert-parallel, 2 experts (= 8192 contiguous sorted tokens) per
core. Host-side dispatch hands each core its token block transposed
(feature-major, fp8 e4m3) plus its 2 experts' weights (fp8 e4m3, pre-scaled
x128 so w_std 0.0054 lands in e4m3's normal range) and bias pre-broadcast to
128 partitions (fp32, pre-scaled x128 to match).

Device pipeline per core: 1024 fp8 DoubleRow matmuls (K=256 per instruction,
2x PE throughput vs fp16) accumulating in fp32 PSUM (t-on-partitions x
o-free tiles, contraction over 4 k-pair blocks), DVE bias add (in fp32 x128
domain, fp16 out), ACT Silu with scale=1/128 folding the weight scale back
out (fp16 out), fp16 store. x loads ride the SP HWDGE ring; weight loads and
output stores ride the ACT HWDGE ring.
"""

import ml_dtypes
import numpy as np

import concourse.bass as bass
import concourse.mybir as mybir
import concourse.tile as tile
from concourse.bass_utils import run_bass_kernel_spmd

N_CORES = 8
N_TOKENS = 65536
D_IN = 1024
D_OUT = 2048
N_EXPERTS = 16
E_PER_CORE = N_EXPERTS // N_CORES        # 2
TOK_PER_CORE = N_TOKENS // N_CORES       # 8192
TOK_PER_EXPERT = N_TOKENS // N_EXPERTS   # 4096

P = 128
KB = D_IN // P            # 8 contraction blocks
TS = 512                  # token stripe
OB = 512                  # out-feature block (one PSUM bank)
N_OB = D_OUT // OB        # 4
N_TSUB = TS // P          # 4
STRIPES_PER_EXPERT = TOK_PER_EXPERT // TS  # 8

F32 = mybir.dt.float32
F16 = mybir.dt.float16
F8 = mybir.dt.float8e4
NP_F8 = ml_dtypes.float8_e4m3
W_SCALE = 128.0  # lifts w_std ~0.0054 out of e4m3 subnormal territory
KH_G = 2          # kb per W tile = one DoubleRow k-pair
NWT_G = KB // KH_G  # 4 W tiles per expert


def _split_multi_waits(nc):
    """This container's walrus encodes at most ONE sync-wait per instruction;
    hoist extras onto single-wait NoOps inserted just before, same engine."""
    for fn in nc.m.functions:
        for bb in fn.blocks:
            insts = list(bb.instructions)
            out = []
            dirty = False
            for inst in insts:
                si = inst.sync_info
                waits = list(si.on_wait) if si and si.on_wait else []
                if len(waits) > 1:
                    dirty = True
                    for j, w in enumerate(waits[:-1]):
                        nop = mybir.InstNoOp(
                            name=f"{inst.name}-prewait{j}", ins=[], outs=[]
                        )
                        nop.engine = inst.engine
                        nop.sync_info = mybir.SyncInfo(on_wait=[w], on_update=[])
                        out.append(nop)
                    inst.sync_info = mybir.SyncInfo(
                        on_wait=[waits[-1]],
                        on_update=list(si.on_update) if si.on_update else [],
                    )
                out.append(inst)
            if dirty:
                bb.instructions = out


N_STRIPES = E_PER_CORE * STRIPES_PER_EXPERT  # 16


def build_kernel():
    nc = bass.Bass()
    # xt pre-striped on host: [stripe, partition, kb, t] so each stripe loads
    # with 8KB-contiguous per-partition lines
    xt = nc.dram_tensor("xt", [N_STRIPES, P, KB, TS], F8, kind="ExternalInput")
    # w pre-packed on host into the exact sbuf tile layout [e, h, p, os, kh, o']
    # so W DMAs are fully contiguous per-partition reads with 4KB (full-tile)
    # or 2KB (os-half) elements — the naive "(kb p) o" rearrange reads
    # scattered 1-2KB chunks at a fraction of the per-queue bandwidth
    # one pack per expert in sbuf tile layout [p, os, h, oc, kh, o'']: a full
    # os-half (all 4 h tiles, 1MB) is ONE contiguous transfer — cold DMAs
    # cost ~4-5us nearly independent of size, so the ramp wants FEW, LARGE
    # transfers
    w = nc.dram_tensor(
        "w", [E_PER_CORE, P, 2, NWT_G, 2, KH_G, D_OUT // 4], F8,
        kind="ExternalInput",
    )
    # bias pre-broadcast on host in fp16 (512KB/expert; fp32 was 1MB of
    # redundant DMA sitting in front of ramp-critical W slices, and the
    # on-device partition_broadcast op doesn't encode in this toolchain)
    bb = nc.dram_tensor("bb", [E_PER_CORE, P, D_OUT], F16, kind="ExternalInput")
    y = nc.dram_tensor("y", [TOK_PER_CORE, D_OUT], F16, kind="ExternalOutput")

    with tile.TileContext(nc) as tc:
        with (
            tc.tile_pool(name="persist", bufs=1) as persist,
            tc.tile_pool(name="xp", bufs=3) as xp,
            tc.tile_pool(name="outs", bufs=4) as outs,
            tc.tile_pool(name="psum", bufs=8, space="PSUM") as psump,
        ):
            x16_tiles = {}
            x16_tiles[0] = xp.tile([P, KB, TS], F8, tag="x16", name="x16_s0")
            nc.sync.dma_start(x16_tiles[0][:], xt[0])

            KH = KH_G
            NWT = NWT_G
            b_sb = [
                persist.tile([P, D_OUT], F16, name=f"bias_{e}")
                for e in range(E_PER_CORE)
            ]
            # one W tile per expert: [p, os, h, oc, kh, o'']; an os-half is a
            # contiguous 8KB-per-partition run = one 1MB DMA
            w16 = [
                persist.tile(
                    [P, 2, NWT, 2, KH, D_OUT // 4], F8, name=f"w16_{e}"
                )
                for e in range(E_PER_CORE)
            ]

            half = D_OUT // 2

            def load_expert0():
                # expert 0 gates the ramp: its whole os0 (all h, 1MB — what
                # the first 8 groups need) arrives as ONE scalar-ring
                # transfer, then os1. Bias first-half rides sync behind x0
                # (first DVE read comes ~2us after the first matmul, with 8
                # PSUM banks of runway). gpsimd's ring has proven ~2x slower
                # when cold, so it only carries the late bias half.
                nc.scalar.dma_start(w16[0][:, 0], w[0, :, 0])
                nc.sync.dma_start(b_sb[0][:, :half], bb[0][:, :half])
                nc.scalar.dma_start(w16[0][:, 1], w[0, :, 1])
                nc.gpsimd.dma_start(b_sb[0][:, half:], bb[0][:, half:])

            def load_expert1():
                # mid-flight on warm queues, off the critical path
                nc.gpsimd.dma_start(b_sb[1][:], bb[1])
                nc.scalar.dma_start(w16[1][:, 0], w[1, :, 0])
                nc.sync.dma_start(w16[1][:, 1], w[1, :, 1])

            load_expert0()

            # PE warmup: dummy DoubleRow matmuls on a memset scratch tile so
            # the tensor engine is at full p-state clock (not the 1.2GHz ramp
            # tier) by the time the critical preload lands; also converts the
            # ~5us data-starved head into busy time
            wu = persist.tile([P, 2, OB], F8, name="warmup")
            nc.vector.memset(wu[:], 0)
            wu_ps = psump.tile([P, OB], F32, tag="ps")
            N_WU = 15
            for i in range(N_WU):
                nc.tensor.matmul(
                    wu_ps[:],
                    lhsT=wu[:, :, 0:P],
                    rhs=wu[:],
                    start=(i == 0),
                    stop=(i == N_WU - 1),
                    perf_mode=mybir.MatmulPerfMode.DoubleRow,
                )

            def mm_group(x16, e, tsub, ob, ps):
                os_, oc = divmod(ob, 2)
                for h in range(NWT):
                    # DoubleRow: K=256 (one kb pair) per matmul; queue q=h%2
                    # holds h at slot h'=h//2
                    nc.tensor.matmul(
                        ps[:],
                        lhsT=x16[:, 2 * h:2 * h + 2, tsub * P:(tsub + 1) * P],
                        rhs=w16[e][:, os_, h, oc],
                        start=(h == 0),
                        stop=(h == NWT - 1),
                        perf_mode=mybir.MatmulPerfMode.DoubleRow,
                    )

            # stripe 0, ob-major: all os0 groups first so the PE ramp only
            # waits on the first half of expert-0's weights; per-half silu +
            # store keeps downstream engines streaming during the ramp
            for os_ in range(2):
                for tsub in range(N_TSUB):
                    yp = outs.tile([P, half], F16, tag="ypreh")
                    ya = outs.tile([P, half], F16, tag="yacth")
                    for oc in range(2):
                        ob = os_ * 2 + oc
                        ps = psump.tile([P, OB], F32, tag="ps")
                        mm_group(x16_tiles[0], 0, tsub, ob, ps)
                        nc.vector.tensor_tensor(
                            yp[:, oc * OB:(oc + 1) * OB], ps[:],
                            b_sb[0][:, ob * OB:(ob + 1) * OB],
                            mybir.AluOpType.add,
                        )
                    nc.scalar.activation(
                        ya[:], yp[:],
                        mybir.ActivationFunctionType.Silu,
                        scale=1.0 / W_SCALE,
                    )
                    [nc.gpsimd, nc.sync, nc.scalar][
                        (os_ * N_TSUB + tsub) % 3
                    ].dma_start(
                        y[tsub * P:(tsub + 1) * P,
                          os_ * half:(os_ + 1) * half],
                        ya[:],
                    )

            for e in range(E_PER_CORE):
                for s in range(STRIPES_PER_EXPERT):
                    g = e * STRIPES_PER_EXPERT + s
                    if g == 0:
                        continue  # handled above, ob-major
                    t0 = g * TS
                    x16 = xp.tile([P, KB, TS], F8, tag="x16", name="x16")
                    nc.sync.dma_start(x16[:], xt[g])

                    if g == N_STRIPES - 1:
                        # final stripe: per os-half silu + 256KB stores,
                        # spread over all rings, so the post-matmul chain and
                        # queue backlogs drain fast
                        for tsub in range(N_TSUB):
                            for os_ in range(2):
                                yp = outs.tile([P, half], F16, tag="ypreh")
                                ya = outs.tile([P, half], F16, tag="yacth")
                                for oc in range(2):
                                    ob = os_ * 2 + oc
                                    ps = psump.tile([P, OB], F32, tag="ps")
                                    mm_group(x16, e, tsub, ob, ps)
                                    nc.vector.tensor_tensor(
                                        yp[:, oc * OB:(oc + 1) * OB], ps[:],
                                        b_sb[e][:, ob * OB:(ob + 1) * OB],
                                        mybir.AluOpType.add,
                                    )
                                nc.scalar.activation(
                                    ya[:], yp[:],
                                    mybir.ActivationFunctionType.Silu,
                                    scale=1.0 / W_SCALE,
                                )
                                # skip the gpsimd ring here: its drain is the
                                # tail's long pole
                                [nc.sync, nc.scalar][
                                    (tsub * 2 + os_) % 2
                                ].dma_start(
                                    y[t0 + tsub * P:t0 + (tsub + 1) * P,
                                      os_ * half:(os_ + 1) * half],
                                    ya[:],
                                )
                        continue

                    for tsub in range(N_TSUB):
                        store_eng = [nc.gpsimd, nc.sync, nc.scalar][
                            (g * N_TSUB + tsub) % 3
                        ]
                        y_pre = outs.tile([P, D_OUT], F16, tag="ypre")
                        y_act = outs.tile([P, D_OUT], F16, tag="yact")
                        for ob in range(N_OB):
                            ps = psump.tile([P, OB], F32, tag="ps")
                            mm_group(x16, e, tsub, ob, ps)
                            # bias add in the x128 domain (bias pre-scaled on
                            # host); fp16 out is exact enough at |v|<~700
                            nc.vector.tensor_tensor(
                                y_pre[:, ob * OB:(ob + 1) * OB], ps[:],
                                b_sb[e][:, ob * OB:(ob + 1) * OB],
                                mybir.AluOpType.add,
                            )
                        # one fused silu per 2048-wide tile amortizes the
                        # ~300ns fixed ACT cost; scale folds the x128
                        # weight scale back out before the nonlinearity
                        nc.scalar.activation(
                            y_act[:], y_pre[:],
                            mybir.ActivationFunctionType.Silu,
                            scale=1.0 / W_SCALE,
                        )
                        store_eng.dma_start(
                            y[t0 + tsub * P:t0 + (tsub + 1) * P, :], y_act[:]
                        )
                    if g == 1:
                        # after g==1 so expert 1's 1MB on the sync ring sits
                        # behind the already-enqueued x1/x2 prefetches
                        load_expert1()

    _split_multi_waits(nc)
    return nc


_NC_CACHE = None


def _get_nc():
    global _NC_CACHE
    if _NC_CACHE is None:
        _NC_CACHE = build_kernel()
    return _NC_CACHE


def _in_maps(sorted_features, routing_matrix, routing_bias):
    maps = []
    for c in range(N_CORES):
        rows = slice(c * TOK_PER_CORE, (c + 1) * TOK_PER_CORE)
        es = slice(c * E_PER_CORE, (c + 1) * E_PER_CORE)
        # [stripe, partition, kb, t]: element (s,p,kb,t) = X_c[s*TS+t, kb*P+p]
        xt_c = np.ascontiguousarray(
            sorted_features[rows]
            .reshape(N_STRIPES, TS, KB, P)
            .transpose(0, 3, 2, 1)
            .astype(NP_F8)
        )
        # pack into the device tile layout [e, p, os, h, oc, kh, o'']:
        # kin = (h*KH+kh)*128 + p, o = os*1024 + oc*512 + o''
        w_c = np.ascontiguousarray(
            (routing_matrix[:, :, es].transpose(2, 0, 1) * W_SCALE)
            .astype(NP_F8)
            .reshape(E_PER_CORE, NWT_G, KH_G, P, 2, 2, D_OUT // 4)
            .transpose(0, 3, 4, 1, 5, 2, 6)
        )
        # bias enters the DVE add in the x128 domain: silu((ps + S*b)/S);
        # fp16 is exact to ~2^-11 relative, far under the fp8 matmul noise
        b_c = np.ascontiguousarray(
            np.broadcast_to(
                (routing_bias[:, es].T * W_SCALE)[:, None, :],
                (E_PER_CORE, P, D_OUT),
            ).astype(np.float16)
        )
        maps.append({"xt": xt_c, "w": w_c, "bb": b_c})
    return maps


def run(sorted_features, routing_matrix, routing_bias, **run_kwargs):
    nc = _get_nc()
    maps = _in_maps(sorted_features, routing_matrix, routing_bias)
    res = run_bass_kernel_spmd(nc, maps, core_ids=list(range(N_CORES)), **run_kwargs)
    out = np.concatenate(
        [res.results[c]["y"].astype(np.float32) for c in range(N_CORES)], axis=0
    )
    return out, res


def kernel(sorted_features, expert_ids_sorted, routing_matrix, routing_bias):
    assert sorted_features.shape == (N_TOKENS, D_IN)
    assert routing_matrix.shape == (D_IN, D_OUT, N_EXPERTS)
    assert routing_bias.shape == (D_OUT, N_EXPERTS)
    out, _ = run(
        np.asarray(sorted_features, dtype=np.float32),
        np.asarray(routing_matrix, dtype=np.float32),
        np.asarray(routing_bias, dtype=np.float32),
    )
    return out

